# revision 6
# baseline (speedup 1.0000x reference)
"""Trainium2 Bass kernel for nn_DCT_Features (dense_cnn).

Math: everything before the LeakyReLU is linear, so the whole module
(3D DCT-II -> mean over dct bins -> per-subwindow full-volume Conv3d)
collapses to one GEMM per subwindow:

  out[b, s*128+k] = LeakyReLU( sum_{n,phi} x[b, s, n, phi] * Weff[s, phi, k] + conv_b[s, k] )

with the mean's 1/2 folded into
  Weff[s, (t,h,w), k] = 0.5 * sum_{f,g,j} conv_w[s,k,f,g,j] Ct[f,t] Ch[g,h] Cw[j,w]

Sharding: pure data parallel over batch, 8 cores x 512 rows; Weff/bias
replicated. The kernel is DMA-bound (all transfers serialize on HBM at
~360 B/ns), so precision is chosen per tensor to minimize bytes within
the 2e-2 error gate: x in fp8 e3m4 (4 mantissa bits; range +-15.5 covers
the N(0,1) input; measured rel err 1.4e-2), Weff/bias in bf16, output in
bf16. Host-side marshaling lays each core's shard out feature-major
([s, kt, p, n, b]) and converts dtype; no input arithmetic on host.

Per core: DMA x tile (fp8) -> matmul accumulate in fp32 PSUM (kout on
partitions, batch on free; the two dct bins contract against the same
weight tile via two matmuls, except a few k-tiles per chunk whose bins
are presummed on DVE to keep PE comfortably under the DMA roofline;
bias applied via a K=1 matmul against a memset ones row) -> exact
2-op LeakyReLU on DVE -> DMA out in bf16 (still [kout, batch]; host
upcasts + un-transposes while gathering the 8 shards).
"""

import os
from contextlib import ExitStack

import numpy as np
import ml_dtypes

import concourse.bass as bass
import concourse.tile as tile
from concourse import bacc, mybir
from concourse.bass_utils import run_bass_kernel_spmd

# Static problem config (hardcoded per contract)
B_FULL = 4096
N_CORES = 8
B_CORE = B_FULL // N_CORES      # 512 batch rows per core
N_SW = 2                        # subwindows
DCT_NBINS = 2
NDCT = 32                       # freqs per subwindow
H = W = 8
KF = NDCT * H * W               # 2048 contraction dim per subwindow per bin
KT = KF // 128                  # 16 k-tiles
KOUT = 128                      # output channels per subwindow
SLOPE = 0.001

W_COLS = N_SW * KT * KOUT       # 4096 weight columns
BIAS_COL = W_COLS               # bias block: row 0 of cols [4096, 4096+256)
WB_COLS = W_COLS + N_SW * KOUT  # 4352

_CACHE = {}
LAST_RESULT = None


def _dct_mat(N):
    n = np.arange(N)
    k = np.arange(N)[:, None]
    return 2.0 * np.cos(np.pi * (2 * n + 1) * k / (2 * N))  # [k, n], float64


def _fold_weights(conv_w, conv_b):
    """Fold DCT matrices + mean into the conv weights (float64 host math)."""
    cw = np.asarray(conv_w, np.float64)          # [s, k, f, g, j]
    Ct = _dct_mat(NDCT)                          # [f, t]
    Ch = _dct_mat(H)                             # [g, h]
    Cw = _dct_mat(W)                             # [j, w]
    we = np.einsum("skfgj,ft,gh,jw->sthwk", cw, Ct, Ch, Cw) * 0.5
    we = we.reshape(N_SW, KF, KOUT)              # [s, phi, k]
    # SBUF layout: w_sb[p, (s*KT+kt)*128 + k] = we[s, kt*128+p, k];
    # bias rides in row 0 of the trailing 256 columns.
    wb = np.zeros((128, WB_COLS), np.float64)
    wb[:, :W_COLS] = (
        we.reshape(N_SW, KT, 128, KOUT).transpose(2, 0, 1, 3).reshape(128, W_COLS)
    )
    wb[0, BIAS_COL:] = np.asarray(conv_b, np.float64).reshape(-1)
    return np.ascontiguousarray(wb.astype(ml_dtypes.bfloat16))


def _shard_x(x):
    """Marshal x into per-core feature-major fp8(e3m4) tiles.

    Returns per-core arrays of shape [N_SW*KT*128, DCT_NBINS*B_CORE] where
    row (s*KT+kt)*128+p, column n*B_CORE+b holds x[c*B_CORE+b, f] with
    f = s*4096 + n*2048 + kt*128 + p.
    """
    X = np.asarray(x, np.float32).reshape(B_FULL, N_SW * DCT_NBINS * KF)
    shards = []
    for c in range(N_CORES):
        v = X[c * B_CORE : (c + 1) * B_CORE].reshape(B_CORE, N_SW, DCT_NBINS, KT, 128)
        p = v.transpose(1, 3, 4, 2, 0)  # [s, kt, p, n, b]
        shards.append(
            np.ascontiguousarray(p)
            .reshape(N_SW * KT * 128, DCT_NBINS * B_CORE)
            .astype(ml_dtypes.float8_e3m4)
        )
    return shards


CHUNK_KT = 4  # k-tiles per x DMA (0.5 MiB fp8 transfers, near HBM-rate)


def _chunk_plan(s):
    """(kt_start, n_kt) DMA chunks for subwindow s. Large chunks for DMA
    efficiency; the last-processed subwindow tapers so less serial work
    trails the final DMA (shorter kernel tail). The first chunk is small
    so PE can start working early (it is a near co-bottleneck)."""
    if s == 0:
        return [(0, 2), (2, 2), (4, 4), (8, 4), (12, 4)]
    return [(0, 4), (4, 4), (8, 4), (12, 2), (14, 1), (15, 1)]


def _presum_this(kt):
    """k-tiles whose dct bins are presummed on DVE (1 matmul instead of 2):
    offloads PE work to the otherwise idle DVE so PE stays under the DMA
    roofline. The tail tiles (kt >= 14) stay on PE to keep the DVE hop out
    of the critical path after the final x chunk lands."""
    return kt % 2 == 1 and kt < 14


def _build_program():
    nc = bacc.Bacc("TRN2", target_bir_lowering=False, debug=False, num_devices=N_CORES)
    f32 = mybir.dt.float32
    bf16 = mybir.dt.bfloat16
    fp8 = mybir.dt.float8e3
    x_ap = nc.dram_tensor(
        "x", [N_SW * KT * 128, DCT_NBINS * B_CORE], fp8, kind="ExternalInput"
    ).ap()
    w_ap = nc.dram_tensor("w", [128, WB_COLS], bf16, kind="ExternalInput").ap()
    # output stays transposed [s*128+k, b]; host upcasts + un-transposes
    out_ap = nc.dram_tensor("out", [N_SW * KOUT, B_CORE], bf16, kind="ExternalOutput").ap()

    with tile.TileContext(nc) as tc, ExitStack() as ctx:
        const = ctx.enter_context(tc.tile_pool(name="const", bufs=1))
        x_pool = ctx.enter_context(tc.tile_pool(name="xp", bufs=6))
        y_pool = ctx.enter_context(tc.tile_pool(name="yp", bufs=4))
        osb_pool = ctx.enter_context(tc.tile_pool(name="osb", bufs=4))
        pout_pool = ctx.enter_context(tc.tile_pool(name="pout", bufs=2, space="PSUM"))

        w_sb = const.tile([128, WB_COLS], bf16)
        ones = const.tile([1, B_CORE], bf16)
        nc.gpsimd.memset(ones[:], 1.0)
        # bias first (tiny: 1 descriptor) on the ACT HWDGE queue so the s=0
        # bias matmul starts early and PE begins its p-state ramp. Weights
        # stream in 4kt-sized pieces interleaved with the x chunks on the
        # SP/ACT queues, each arriving ahead of need.
        nc.scalar.dma_start(out=w_sb[0:1, BIAS_COL:], in_=w_ap[0:1, BIAS_COL:])

        x_re = x_ap.rearrange("(t p) f -> p t f", p=128)  # [128, 32, 1024]

        for s in range(N_SW):
            psum_out = pout_pool.tile([KOUT, B_CORE], f32)
            # bias via K=1 matmul against the ones row: starts the psum
            # accumulation group and keeps bias-add off the DVE epilogue.
            nc.tensor.matmul(
                psum_out[:],
                lhsT=w_sb[0:1, bass.ds(BIAS_COL + s * KOUT, KOUT)],
                rhs=ones[:],
                start=True,
                stop=False,
            )
            for g, (kt0, nkt) in enumerate(_chunk_plan(s)):
                # weight piece for this kt range, one chunk ahead of its use
                wcols = bass.ds((s * KT + kt0) * KOUT, nkt * KOUT)
                xab = x_pool.tile([128, CHUNK_KT, DCT_NBINS * B_CORE], fp8)
                # alternate the two HWDGE queues (SP / ACT) for pipelined
                # descriptor generation while transfers serialize on HBM
                dma_eng = nc.sync if g % 2 == 0 else nc.scalar
                w_eng = nc.scalar if g % 2 == 0 else nc.sync
                w_eng.dma_start(out=w_sb[:, wcols], in_=w_ap[:, wcols])
                dma_eng.dma_start(
                    out=xab[:, 0:nkt, :], in_=x_re[:, bass.ds(s * KT + kt0, nkt), :]
                )
                for j in range(nkt):
                    kt = kt0 + j
                    lhsT = w_sb[:, bass.ts(s * KT + kt, 128)]
                    last = kt == KT - 1
                    if _presum_this(kt):
                        y = y_pool.tile([128, B_CORE], bf16)
                        nc.vector.tensor_add(
                            y[:], xab[:, j, 0:B_CORE], xab[:, j, B_CORE:]
                        )
                        nc.tensor.matmul(
                            psum_out[:], lhsT=lhsT, rhs=y[:], start=False, stop=last
                        )
                    else:
                        for n in range(DCT_NBINS):
                            nc.tensor.matmul(
                                psum_out[:],
                                lhsT=lhsT,
                                rhs=xab[:, j, bass.ts(n, B_CORE)],
                                start=False,
                                stop=last and n == DCT_NBINS - 1,
                            )
            # epilogue: exact LeakyReLU as max(y, SLOPE*y) on DVE; bias
            # already in psum. Halved along batch so the first output DMA
            # starts early (GPSIMD cannot read PSUM, so both halves stay on
            # DVE). Out DMAs ride SP/ACT behind the x chunks (the epilogue
            # data is always ready before the queue drains).
            for h in range(2):
                hb = bass.ts(h, B_CORE // 2)
                tl = osb_pool.tile([KOUT, B_CORE // 2], f32, tag="tl", name=f"tl_{s}_{h}")
                nc.vector.tensor_scalar_mul(tl[:], psum_out[:, hb], SLOPE)
                osb = osb_pool.tile([KOUT, B_CORE // 2], bf16, tag="osb", name=f"osb_{s}_{h}")
                nc.vector.tensor_max(osb[:], psum_out[:, hb], tl[:])
                eng = nc.sync if h == 0 else nc.scalar
                eng.dma_start(out=out_ap[bass.ts(s, KOUT), hb], in_=osb[:])

    nc.compile()
    return nc


def _get_program():
    if "nc" not in _CACHE:
        _CACHE["nc"] = _build_program()
    return _CACHE["nc"]


def kernel(x, conv_w, conv_b):
    global LAST_RESULT
    shards = _shard_x(x)
    wb_host = _fold_weights(conv_w, conv_b)

    nc = _get_program()
    in_maps = [{"x": shards[c], "w": wb_host} for c in range(N_CORES)]
    trace = bool(int(os.environ.get("DCT_TRACE", "0")))
    res = run_bass_kernel_spmd(nc, in_maps, list(range(N_CORES)), trace=trace)
    LAST_RESULT = res
    # per-core output is [s*128+k, b] bf16; upcast + un-transpose during gather
    out = np.concatenate(
        [
            np.ascontiguousarray(np.asarray(res.results[c]["out"], np.float32).T)
            for c in range(N_CORES)
        ],
        axis=0,
    )
    return out


# revision 7
# speedup vs baseline: 1.0048x; 1.0048x over previous
"""Trainium2 Bass kernel for nn_DCT_Features (dense_cnn).

Math: everything before the LeakyReLU is linear, so the whole module
(3D DCT-II -> mean over dct bins -> per-subwindow full-volume Conv3d)
collapses to one GEMM per subwindow:

  out[b, s*128+k] = LeakyReLU( sum_{n,phi} x[b, s, n, phi] * Weff[s, phi, k] + conv_b[s, k] )

with the mean's 1/2 folded into
  Weff[s, (t,h,w), k] = 0.5 * sum_{f,g,j} conv_w[s,k,f,g,j] Ct[f,t] Ch[g,h] Cw[j,w]

Sharding: pure data parallel over batch, 8 cores x 512 rows; Weff/bias
replicated. The kernel is DMA-bound (all transfers serialize on HBM at
~360 B/ns), so precision is chosen per tensor to minimize bytes within
the 2e-2 error gate: x in fp8 e3m4 (4 mantissa bits; range +-15.5 covers
the N(0,1) input; measured rel err 1.4e-2), Weff/bias in bf16, output in
bf16. Host-side marshaling lays each core's shard out feature-major
([s, kt, p, n, b]) and converts dtype; no input arithmetic on host.

Per core: DMA x tile (fp8) -> matmul accumulate in fp32 PSUM (kout on
partitions, batch on free; the two dct bins contract against the same
weight tile via two matmuls, except a few k-tiles per chunk whose bins
are presummed on DVE to keep PE comfortably under the DMA roofline;
bias applied via a K=1 matmul against a memset ones row) -> exact
2-op LeakyReLU on DVE -> DMA out in bf16 (still [kout, batch]; host
upcasts + un-transposes while gathering the 8 shards).
"""

import os
from contextlib import ExitStack

import numpy as np
import ml_dtypes

import concourse.bass as bass
import concourse.tile as tile
from concourse import bacc, mybir
from concourse.bass_utils import run_bass_kernel_spmd

# Static problem config (hardcoded per contract)
B_FULL = 4096
N_CORES = 8
B_CORE = B_FULL // N_CORES      # 512 batch rows per core
N_SW = 2                        # subwindows
DCT_NBINS = 2
NDCT = 32                       # freqs per subwindow
H = W = 8
KF = NDCT * H * W               # 2048 contraction dim per subwindow per bin
KT = KF // 128                  # 16 k-tiles
KOUT = 128                      # output channels per subwindow
SLOPE = 0.001

W_COLS = N_SW * KT * KOUT       # 4096 weight columns
BIAS_COLS = N_SW * KOUT         # bias block first: row 0 of cols [0, 256)
W0 = BIAS_COLS                  # weight columns start here
WB_COLS = W_COLS + BIAS_COLS    # 4352

_CACHE = {}
LAST_RESULT = None


def _dct_mat(N):
    n = np.arange(N)
    k = np.arange(N)[:, None]
    return 2.0 * np.cos(np.pi * (2 * n + 1) * k / (2 * N))  # [k, n], float64


def _fold_weights(conv_w, conv_b):
    """Fold DCT matrices + mean into the conv weights (float64 host math)."""
    cw = np.asarray(conv_w, np.float64)          # [s, k, f, g, j]
    Ct = _dct_mat(NDCT)                          # [f, t]
    Ch = _dct_mat(H)                             # [g, h]
    Cw = _dct_mat(W)                             # [j, w]
    we = np.einsum("skfgj,ft,gh,jw->sthwk", cw, Ct, Ch, Cw) * 0.5
    we = we.reshape(N_SW, KF, KOUT)              # [s, phi, k]
    # SBUF layout: w_sb[p, (s*KT+kt)*128 + k] = we[s, kt*128+p, k];
    # bias rides in row 0 of the trailing 256 columns.
    wb = np.zeros((128, WB_COLS), np.float64)
    wb[:, W0:] = (
        we.reshape(N_SW, KT, 128, KOUT).transpose(2, 0, 1, 3).reshape(128, W_COLS)
    )
    wb[0, :BIAS_COLS] = np.asarray(conv_b, np.float64).reshape(-1)
    return np.ascontiguousarray(wb.astype(ml_dtypes.bfloat16))


def _shard_x(x):
    """Marshal x into per-core feature-major fp8(e3m4) tiles.

    Returns per-core arrays of shape [N_SW*KT*128, DCT_NBINS*B_CORE] where
    row (s*KT+kt)*128+p, column n*B_CORE+b holds x[c*B_CORE+b, f] with
    f = s*4096 + n*2048 + kt*128 + p.
    """
    X = np.asarray(x, np.float32).reshape(B_FULL, N_SW * DCT_NBINS * KF)
    shards = []
    for c in range(N_CORES):
        v = X[c * B_CORE : (c + 1) * B_CORE].reshape(B_CORE, N_SW, DCT_NBINS, KT, 128)
        p = v.transpose(1, 3, 4, 2, 0)  # [s, kt, p, n, b]
        shards.append(
            np.ascontiguousarray(p)
            .reshape(N_SW * KT * 128, DCT_NBINS * B_CORE)
            .astype(ml_dtypes.float8_e3m4)
        )
    return shards


CHUNK_KT = 4  # k-tiles per x DMA (0.5 MiB fp8 transfers, near HBM-rate)


def _chunk_plan(s):
    """(kt_start, n_kt) DMA chunks for subwindow s. Large chunks for DMA
    efficiency; the last-processed subwindow tapers so less serial work
    trails the final DMA (shorter kernel tail). The first chunk is small
    so PE can start working early (it is a near co-bottleneck)."""
    if s == 0:
        return [(0, 4), (4, 4), (8, 4), (12, 4)]
    return [(0, 4), (4, 4), (8, 4), (12, 2), (14, 1), (15, 1)]


def _presum_this(kt):
    """k-tiles whose dct bins are presummed on DVE (1 matmul instead of 2):
    offloads PE work to the otherwise idle DVE so PE stays under the DMA
    roofline. The tail tiles (kt >= 14) stay on PE to keep the DVE hop out
    of the critical path after the final x chunk lands."""
    return kt % 2 == 1 and kt < 14


def _build_program():
    nc = bacc.Bacc("TRN2", target_bir_lowering=False, debug=False, num_devices=N_CORES)
    f32 = mybir.dt.float32
    bf16 = mybir.dt.bfloat16
    fp8 = mybir.dt.float8e3
    x_ap = nc.dram_tensor(
        "x", [N_SW * KT * 128, DCT_NBINS * B_CORE], fp8, kind="ExternalInput"
    ).ap()
    w_ap = nc.dram_tensor("w", [128, WB_COLS], bf16, kind="ExternalInput").ap()
    # output stays transposed [s*128+k, b]; host upcasts + un-transposes
    out_ap = nc.dram_tensor("out", [N_SW * KOUT, B_CORE], bf16, kind="ExternalOutput").ap()

    with tile.TileContext(nc) as tc, ExitStack() as ctx:
        const = ctx.enter_context(tc.tile_pool(name="const", bufs=1))
        x_pool = ctx.enter_context(tc.tile_pool(name="xp", bufs=6))
        y_pool = ctx.enter_context(tc.tile_pool(name="yp", bufs=4))
        osb_pool = ctx.enter_context(tc.tile_pool(name="osb", bufs=4))
        pout_pool = ctx.enter_context(tc.tile_pool(name="pout", bufs=2, space="PSUM"))

        w_sb = const.tile([128, WB_COLS], bf16)
        ones = const.tile([1, B_CORE], bf16)
        nc.gpsimd.memset(ones[:], 1.0)
        # Three weight loads: [bias | s0 kt0-3] small and first on SP HWDGE
        # (so PE can start as soon as the first x chunk lands), the rest via
        # the Pool/SWDGE path — descriptor generation for the bulk weights
        # happens on the otherwise idle GPSIMD engine instead of the shared
        # HWDGE, which the 8 x chunks + 4 out DMAs already keep busy.
        wsplit = W0 + 4 * KOUT
        nc.sync.dma_start(out=w_sb[:, 0:wsplit], in_=w_ap[:, 0:wsplit])
        nc.gpsimd.dma_start(
            out=w_sb[:, wsplit : W0 + KT * KOUT], in_=w_ap[:, wsplit : W0 + KT * KOUT]
        )
        nc.gpsimd.dma_start(
            out=w_sb[:, W0 + KT * KOUT :], in_=w_ap[:, W0 + KT * KOUT :]
        )

        x_re = x_ap.rearrange("(t p) f -> p t f", p=128)  # [128, 32, 1024]

        out_dmas = []  # (engine, out slice, sbuf tile): issued after all x

        for s in range(N_SW):
            psum_out = pout_pool.tile([KOUT, B_CORE], f32)
            # bias via K=1 matmul against the ones row: starts the psum
            # accumulation group and keeps bias-add off the DVE epilogue.
            nc.tensor.matmul(
                psum_out[:],
                lhsT=w_sb[0:1, bass.ds(s * KOUT, KOUT)],
                rhs=ones[:],
                start=True,
                stop=False,
            )
            for g, (kt0, nkt) in enumerate(_chunk_plan(s)):
                xab = x_pool.tile([128, CHUNK_KT, DCT_NBINS * B_CORE], fp8)
                # alternate the two HWDGE queues (SP / ACT) for pipelined
                # descriptor generation while transfers serialize on HBM
                dma_eng = nc.scalar if (s * 4 + g) % 2 == 0 else nc.sync
                dma_eng.dma_start(
                    out=xab[:, 0:nkt, :], in_=x_re[:, bass.ds(s * KT + kt0, nkt), :]
                )
                for j in range(nkt):
                    kt = kt0 + j
                    lhsT = w_sb[:, bass.ds(W0 + (s * KT + kt) * KOUT, KOUT)]
                    last = kt == KT - 1
                    if _presum_this(kt):
                        y = y_pool.tile([128, B_CORE], bf16)
                        nc.vector.tensor_add(
                            y[:], xab[:, j, 0:B_CORE], xab[:, j, B_CORE:]
                        )
                        nc.tensor.matmul(
                            psum_out[:], lhsT=lhsT, rhs=y[:], start=False, stop=last
                        )
                    else:
                        for n in range(DCT_NBINS):
                            nc.tensor.matmul(
                                psum_out[:],
                                lhsT=lhsT,
                                rhs=xab[:, j, bass.ts(n, B_CORE)],
                                start=False,
                                stop=last and n == DCT_NBINS - 1,
                            )
            # epilogue: exact LeakyReLU as max(y, SLOPE*y) on DVE; bias
            # already in psum. Halved along batch so the first output DMA
            # starts early (GPSIMD cannot read PSUM, so both halves stay on
            # DVE). The DMAs themselves are deferred to the end of each
            # queue's program order — an out DMA queued between x chunks
            # would stall the x stream behind the epilogue's semaphore.
            for h in range(2):
                hb = bass.ts(h, B_CORE // 2)
                tl = osb_pool.tile([KOUT, B_CORE // 2], f32, tag="tl", name=f"tl_{s}_{h}")
                nc.vector.tensor_scalar_mul(tl[:], psum_out[:, hb], SLOPE)
                osb = osb_pool.tile([KOUT, B_CORE // 2], bf16, tag="osb", name=f"osb_{s}_{h}")
                nc.vector.tensor_max(osb[:], psum_out[:, hb], tl[:])
                eng = nc.sync if h == 0 else nc.scalar
                out_dmas.append((eng, out_ap[bass.ts(s, KOUT), hb], osb))

        for eng, dst, osb in out_dmas:
            eng.dma_start(out=dst, in_=osb[:])

    nc.compile()
    return nc


def _get_program():
    if "nc" not in _CACHE:
        _CACHE["nc"] = _build_program()
    return _CACHE["nc"]


def kernel(x, conv_w, conv_b):
    global LAST_RESULT
    shards = _shard_x(x)
    wb_host = _fold_weights(conv_w, conv_b)

    nc = _get_program()
    in_maps = [{"x": shards[c], "w": wb_host} for c in range(N_CORES)]
    trace = bool(int(os.environ.get("DCT_TRACE", "0")))
    res = run_bass_kernel_spmd(nc, in_maps, list(range(N_CORES)), trace=trace)
    LAST_RESULT = res
    # per-core output is [s*128+k, b] bf16; upcast + un-transpose during gather
    out = np.concatenate(
        [
            np.ascontiguousarray(np.asarray(res.results[c]["out"], np.float32).T)
            for c in range(N_CORES)
        ],
        axis=0,
    )
    return out


# revision 8
# speedup vs baseline: 1.0063x; 1.0015x over previous
"""Trainium2 Bass kernel for nn_DCT_Features (dense_cnn).

Math: everything before the LeakyReLU is linear, so the whole module
(3D DCT-II -> mean over dct bins -> per-subwindow full-volume Conv3d)
collapses to one GEMM per subwindow:

  out[b, s*128+k] = LeakyReLU( sum_{n,phi} x[b, s, n, phi] * Weff[s, phi, k] + conv_b[s, k] )

with the mean's 1/2 folded into
  Weff[s, (t,h,w), k] = 0.5 * sum_{f,g,j} conv_w[s,k,f,g,j] Ct[f,t] Ch[g,h] Cw[j,w]

Sharding: pure data parallel over batch, 8 cores x 512 rows; Weff/bias
replicated. The kernel is DMA-bound (all transfers serialize on HBM at
~360 B/ns), so precision is chosen per tensor to minimize bytes within
the 2e-2 error gate: x in fp8 e3m4 (4 mantissa bits; range +-15.5 covers
the N(0,1) input; measured rel err 1.4e-2), Weff/bias in bf16, output in
bf16. Host-side marshaling lays each core's shard out feature-major
([s, kt, p, n, b]) and converts dtype; no input arithmetic on host.

Per core: DMA x tile (fp8) -> matmul accumulate in fp32 PSUM (kout on
partitions, batch on free; the two dct bins contract against the same
weight tile via two matmuls, except a few k-tiles per chunk whose bins
are presummed on DVE to keep PE comfortably under the DMA roofline;
bias applied via a K=1 matmul against a memset ones row) -> exact
2-op LeakyReLU on DVE -> DMA out in bf16 (still [kout, batch]; host
upcasts + un-transposes while gathering the 8 shards).
"""

import os
from contextlib import ExitStack

import numpy as np
import ml_dtypes

import concourse.bass as bass
import concourse.tile as tile
from concourse import bacc, mybir
from concourse.bass_utils import run_bass_kernel_spmd

# Static problem config (hardcoded per contract)
B_FULL = 4096
N_CORES = 8
B_CORE = B_FULL // N_CORES      # 512 batch rows per core
N_SW = 2                        # subwindows
DCT_NBINS = 2
NDCT = 32                       # freqs per subwindow
H = W = 8
KF = NDCT * H * W               # 2048 contraction dim per subwindow per bin
KT = KF // 128                  # 16 k-tiles
KOUT = 128                      # output channels per subwindow
SLOPE = 0.001

W_COLS = N_SW * KT * KOUT       # 4096 weight columns
BIAS_COLS = N_SW * KOUT         # bias block first: row 0 of cols [0, 256)
W0 = BIAS_COLS                  # weight columns start here
WB_COLS = W_COLS + BIAS_COLS    # 4352

_CACHE = {}
LAST_RESULT = None


def _dct_mat(N):
    n = np.arange(N)
    k = np.arange(N)[:, None]
    return 2.0 * np.cos(np.pi * (2 * n + 1) * k / (2 * N))  # [k, n], float64


def _fold_weights(conv_w, conv_b):
    """Fold DCT matrices + mean into the conv weights (float64 host math)."""
    cw = np.asarray(conv_w, np.float64)          # [s, k, f, g, j]
    Ct = _dct_mat(NDCT)                          # [f, t]
    Ch = _dct_mat(H)                             # [g, h]
    Cw = _dct_mat(W)                             # [j, w]
    we = np.einsum("skfgj,ft,gh,jw->sthwk", cw, Ct, Ch, Cw) * 0.5
    we = we.reshape(N_SW, KF, KOUT)              # [s, phi, k]
    # SBUF layout: w_sb[p, (s*KT+kt)*128 + k] = we[s, kt*128+p, k];
    # bias rides in row 0 of the trailing 256 columns.
    wb = np.zeros((128, WB_COLS), np.float64)
    wb[:, W0:] = (
        we.reshape(N_SW, KT, 128, KOUT).transpose(2, 0, 1, 3).reshape(128, W_COLS)
    )
    wb[0, :BIAS_COLS] = np.asarray(conv_b, np.float64).reshape(-1)
    return np.ascontiguousarray(wb.astype(ml_dtypes.bfloat16))


def _shard_x(x):
    """Marshal x into per-core feature-major fp8(e3m4) tiles.

    Returns per-core arrays of shape [N_SW*KT*128, DCT_NBINS*B_CORE] where
    row (s*KT+kt)*128+p, column n*B_CORE+b holds x[c*B_CORE+b, f] with
    f = s*4096 + n*2048 + kt*128 + p.
    """
    X = np.asarray(x, np.float32).reshape(B_FULL, N_SW * DCT_NBINS * KF)
    shards = []
    for c in range(N_CORES):
        v = X[c * B_CORE : (c + 1) * B_CORE].reshape(B_CORE, N_SW, DCT_NBINS, KT, 128)
        p = v.transpose(1, 3, 4, 2, 0)  # [s, kt, p, n, b]
        shards.append(
            np.ascontiguousarray(p)
            .reshape(N_SW * KT * 128, DCT_NBINS * B_CORE)
            .astype(ml_dtypes.float8_e3m4)
        )
    return shards


CHUNK_KT = 4  # k-tiles per x DMA (0.5 MiB fp8 transfers, near HBM-rate)


def _chunk_plan(s):
    """(kt_start, n_kt) DMA chunks for subwindow s. Large chunks for DMA
    efficiency; the last-processed subwindow tapers so less serial work
    trails the final DMA (shorter kernel tail). The first chunk is small
    so PE can start working early (it is a near co-bottleneck)."""
    if s == 0:
        return [(0, 4), (4, 4), (8, 4), (12, 4)]
    return [(0, 4), (4, 4), (8, 4), (12, 2), (14, 1), (15, 1)]


def _presum_this(s, kt):
    """k-tiles whose dct bins are presummed on DVE (1 matmul instead of 2):
    offloads PE work to the otherwise idle DVE while PE is still ramping its
    p-state during s=0. All of s=1 stays on PE: its matmuls must not wait on
    the serial DVE queue (which also runs the s=0 epilogue) — PE consuming
    chunks the moment their semaphore fires is what keeps the kernel tail
    short."""
    return s == 0 and kt % 2 == 1 and kt < 14


def _build_program():
    nc = bacc.Bacc("TRN2", target_bir_lowering=False, debug=False, num_devices=N_CORES)
    f32 = mybir.dt.float32
    bf16 = mybir.dt.bfloat16
    fp8 = mybir.dt.float8e3
    x_ap = nc.dram_tensor(
        "x", [N_SW * KT * 128, DCT_NBINS * B_CORE], fp8, kind="ExternalInput"
    ).ap()
    w_ap = nc.dram_tensor("w", [128, WB_COLS], bf16, kind="ExternalInput").ap()
    # output stays transposed [s*128+k, b]; host upcasts + un-transposes
    out_ap = nc.dram_tensor("out", [N_SW * KOUT, B_CORE], bf16, kind="ExternalOutput").ap()

    with tile.TileContext(nc) as tc, ExitStack() as ctx:
        const = ctx.enter_context(tc.tile_pool(name="const", bufs=1))
        x_pool = ctx.enter_context(tc.tile_pool(name="xp", bufs=6))
        y_pool = ctx.enter_context(tc.tile_pool(name="yp", bufs=4))
        osb_pool = ctx.enter_context(tc.tile_pool(name="osb", bufs=4))
        pout_pool = ctx.enter_context(tc.tile_pool(name="pout", bufs=2, space="PSUM"))

        w_sb = const.tile([128, WB_COLS], bf16)
        ones = const.tile([1, B_CORE], bf16)
        nc.gpsimd.memset(ones[:], 1.0)
        # Three weight loads: [bias | s0 kt0-3] small and first on SP HWDGE
        # (so PE can start as soon as the first x chunk lands), the rest via
        # the Pool/SWDGE path — descriptor generation for the bulk weights
        # happens on the otherwise idle GPSIMD engine instead of the shared
        # HWDGE, which the 8 x chunks + 4 out DMAs already keep busy.
        wsplit = W0 + 4 * KOUT
        nc.sync.dma_start(out=w_sb[:, 0:wsplit], in_=w_ap[:, 0:wsplit])
        nc.gpsimd.dma_start(
            out=w_sb[:, wsplit : W0 + KT * KOUT], in_=w_ap[:, wsplit : W0 + KT * KOUT]
        )
        nc.gpsimd.dma_start(
            out=w_sb[:, W0 + KT * KOUT :], in_=w_ap[:, W0 + KT * KOUT :]
        )

        x_re = x_ap.rearrange("(t p) f -> p t f", p=128)  # [128, 32, 1024]

        out_dmas = []  # (engine, out slice, sbuf tile): issued after all x

        for s in range(N_SW):
            psum_out = pout_pool.tile([KOUT, B_CORE], f32)
            # bias via K=1 matmul against the ones row: starts the psum
            # accumulation group and keeps bias-add off the DVE epilogue.
            nc.tensor.matmul(
                psum_out[:],
                lhsT=w_sb[0:1, bass.ds(s * KOUT, KOUT)],
                rhs=ones[:],
                start=True,
                stop=False,
            )
            for g, (kt0, nkt) in enumerate(_chunk_plan(s)):
                xab = x_pool.tile([128, CHUNK_KT, DCT_NBINS * B_CORE], fp8)
                # alternate the two HWDGE queues (SP / ACT) for pipelined
                # descriptor generation while transfers serialize on HBM
                dma_eng = nc.scalar if (s * 4 + g) % 2 == 0 else nc.sync
                dma_eng.dma_start(
                    out=xab[:, 0:nkt, :], in_=x_re[:, bass.ds(s * KT + kt0, nkt), :]
                )
                for j in range(nkt):
                    kt = kt0 + j
                    lhsT = w_sb[:, bass.ds(W0 + (s * KT + kt) * KOUT, KOUT)]
                    last = kt == KT - 1
                    if _presum_this(s, kt):
                        y = y_pool.tile([128, B_CORE], bf16)
                        nc.vector.tensor_add(
                            y[:], xab[:, j, 0:B_CORE], xab[:, j, B_CORE:]
                        )
                        nc.tensor.matmul(
                            psum_out[:], lhsT=lhsT, rhs=y[:], start=False, stop=last
                        )
                    else:
                        for n in range(DCT_NBINS):
                            nc.tensor.matmul(
                                psum_out[:],
                                lhsT=lhsT,
                                rhs=xab[:, j, bass.ts(n, B_CORE)],
                                start=False,
                                stop=last and n == DCT_NBINS - 1,
                            )
            # epilogue: exact LeakyReLU as max(y, SLOPE*y) on DVE; bias
            # already in psum. Halved along batch so the first output DMA
            # starts early (GPSIMD cannot read PSUM, so both halves stay on
            # DVE). The DMAs themselves are deferred to the end of each
            # queue's program order — an out DMA queued between x chunks
            # would stall the x stream behind the epilogue's semaphore.
            for h in range(2):
                hb = bass.ts(h, B_CORE // 2)
                tl = osb_pool.tile([KOUT, B_CORE // 2], f32, tag="tl", name=f"tl_{s}_{h}")
                nc.vector.tensor_scalar_mul(tl[:], psum_out[:, hb], SLOPE)
                osb = osb_pool.tile([KOUT, B_CORE // 2], bf16, tag="osb", name=f"osb_{s}_{h}")
                nc.vector.tensor_max(osb[:], psum_out[:, hb], tl[:])
                eng = nc.sync if h == 0 else nc.scalar
                out_dmas.append((eng, out_ap[bass.ts(s, KOUT), hb], osb))

        for eng, dst, osb in out_dmas:
            eng.dma_start(out=dst, in_=osb[:])

    nc.compile()
    return nc


def _get_program():
    if "nc" not in _CACHE:
        _CACHE["nc"] = _build_program()
    return _CACHE["nc"]


def kernel(x, conv_w, conv_b):
    global LAST_RESULT
    shards = _shard_x(x)
    wb_host = _fold_weights(conv_w, conv_b)

    nc = _get_program()
    in_maps = [{"x": shards[c], "w": wb_host} for c in range(N_CORES)]
    trace = bool(int(os.environ.get("DCT_TRACE", "0")))
    res = run_bass_kernel_spmd(nc, in_maps, list(range(N_CORES)), trace=trace)
    LAST_RESULT = res
    # per-core output is [s*128+k, b] bf16; upcast + un-transpose during gather
    out = np.concatenate(
        [
            np.ascontiguousarray(np.asarray(res.results[c]["out"], np.float32).T)
            for c in range(N_CORES)
        ],
        axis=0,
    )
    return out


# revision 9
# speedup vs baseline: 1.0119x; 1.0056x over previous
"""Trainium2 Bass kernel for nn_DCT_Features (dense_cnn).

Math: everything before the LeakyReLU is linear, so the whole module
(3D DCT-II -> mean over dct bins -> per-subwindow full-volume Conv3d)
collapses to one GEMM per subwindow:

  out[b, s*128+k] = LeakyReLU( sum_{n,phi} x[b, s, n, phi] * Weff[s, phi, k] + conv_b[s, k] )

with the mean's 1/2 folded into
  Weff[s, (t,h,w), k] = 0.5 * sum_{f,g,j} conv_w[s,k,f,g,j] Ct[f,t] Ch[g,h] Cw[j,w]

Sharding: pure data parallel over batch, 8 cores x 512 rows; Weff/bias
replicated. The kernel is DMA-bound (all transfers serialize on HBM at
~360 B/ns), so precision is chosen per tensor to minimize bytes within
the 2e-2 error gate: x in fp8 e3m4 (4 mantissa bits; range +-15.5 covers
the N(0,1) input; measured rel err 1.4e-2), Weff/bias in bf16, output in
bf16. Host-side marshaling lays each core's shard out feature-major
([s, kt, p, n, b]) and converts dtype; no input arithmetic on host.

Per core: DMA x tile (fp8) -> matmul accumulate in fp32 PSUM (kout on
partitions, batch on free; the two dct bins contract against the same
weight tile via two matmuls, except a few k-tiles per chunk whose bins
are presummed on DVE to keep PE comfortably under the DMA roofline;
bias applied via a K=1 matmul against a memset ones row) -> exact
2-op LeakyReLU on DVE -> DMA out in bf16 (still [kout, batch]; host
upcasts + un-transposes while gathering the 8 shards).
"""

import os
from contextlib import ExitStack

import numpy as np
import ml_dtypes

import concourse.bass as bass
import concourse.tile as tile
from concourse import bacc, mybir
from concourse.bass_utils import run_bass_kernel_spmd

# Static problem config (hardcoded per contract)
B_FULL = 4096
N_CORES = 8
B_CORE = B_FULL // N_CORES      # 512 batch rows per core
N_SW = 2                        # subwindows
DCT_NBINS = 2
NDCT = 32                       # freqs per subwindow
H = W = 8
KF = NDCT * H * W               # 2048 contraction dim per subwindow per bin
KT = KF // 128                  # 16 k-tiles
KOUT = 128                      # output channels per subwindow
SLOPE = 0.001

W_COLS = N_SW * KT * KOUT       # 4096 weight columns
BIAS_COLS = N_SW * KOUT         # bias block first: row 0 of cols [0, 256)
W0 = BIAS_COLS                  # weight columns start here
WB_COLS = W_COLS + BIAS_COLS    # 4352

_CACHE = {}
LAST_RESULT = None


def _dct_mat(N):
    n = np.arange(N)
    k = np.arange(N)[:, None]
    return 2.0 * np.cos(np.pi * (2 * n + 1) * k / (2 * N))  # [k, n], float64


def _fold_weights(conv_w, conv_b):
    """Fold DCT matrices + mean into the conv weights (float64 host math)."""
    cw = np.asarray(conv_w, np.float64)          # [s, k, f, g, j]
    Ct = _dct_mat(NDCT)                          # [f, t]
    Ch = _dct_mat(H)                             # [g, h]
    Cw = _dct_mat(W)                             # [j, w]
    we = np.einsum("skfgj,ft,gh,jw->sthwk", cw, Ct, Ch, Cw) * 0.5
    we = we.reshape(N_SW, KF, KOUT)              # [s, phi, k]
    # SBUF layout: w_sb[p, (s*KT+kt)*128 + k] = we[s, kt*128+p, k];
    # bias rides in row 0 of the trailing 256 columns.
    wb = np.zeros((128, WB_COLS), np.float64)
    wb[:, W0:] = (
        we.reshape(N_SW, KT, 128, KOUT).transpose(2, 0, 1, 3).reshape(128, W_COLS)
    )
    wb[0, :BIAS_COLS] = np.asarray(conv_b, np.float64).reshape(-1)
    return np.ascontiguousarray(wb.astype(ml_dtypes.bfloat16))


def _shard_x(x):
    """Marshal x into per-core feature-major fp8(e3m4) tiles.

    Returns per-core arrays of shape [N_SW*KT*128, DCT_NBINS*B_CORE] where
    row (s*KT+kt)*128+p, column n*B_CORE+b holds x[c*B_CORE+b, f] with
    f = s*4096 + n*2048 + kt*128 + p.
    """
    X = np.asarray(x, np.float32).reshape(B_FULL, N_SW * DCT_NBINS * KF)
    shards = []
    for c in range(N_CORES):
        v = X[c * B_CORE : (c + 1) * B_CORE].reshape(B_CORE, N_SW, DCT_NBINS, KT, 128)
        p = v.transpose(1, 3, 4, 2, 0)  # [s, kt, p, n, b]
        shards.append(
            np.ascontiguousarray(p)
            .reshape(N_SW * KT * 128, DCT_NBINS * B_CORE)
            .astype(ml_dtypes.float8_e3m4)
        )
    return shards


CHUNK_KT = 4  # k-tiles per x DMA (0.5 MiB fp8 transfers, near HBM-rate)


def _chunk_plan(s):
    """(kt_start, n_kt) DMA chunks for subwindow s. Large chunks for DMA
    efficiency; the last-processed subwindow tapers so less serial work
    trails the final DMA (shorter kernel tail). The first chunk is small
    so PE can start working early (it is a near co-bottleneck)."""
    if s == 0:
        return [(0, 2), (2, 2), (4, 4), (8, 4), (12, 4)]
    return [(0, 4), (4, 4), (8, 4), (12, 2), (14, 1), (15, 1)]


def _presum_this(s, kt):
    """k-tiles whose dct bins are presummed on DVE (1 matmul instead of 2):
    offloads PE work to the mostly idle DVE so PE finishes with the DMA
    stream instead of draining a backlog after it. The tail k-tiles of each
    subwindow stay on PE so nothing after the last x chunk waits on the
    serial DVE queue."""
    return kt % 2 == 1 and kt < (14 if s == 0 else 12)


def _build_program():
    nc = bacc.Bacc("TRN2", target_bir_lowering=False, debug=False, num_devices=N_CORES)
    f32 = mybir.dt.float32
    bf16 = mybir.dt.bfloat16
    fp8 = mybir.dt.float8e3
    x_ap = nc.dram_tensor(
        "x", [N_SW * KT * 128, DCT_NBINS * B_CORE], fp8, kind="ExternalInput"
    ).ap()
    w_ap = nc.dram_tensor("w", [128, WB_COLS], bf16, kind="ExternalInput").ap()
    # output stays transposed [s*128+k, b]; host upcasts + un-transposes
    out_ap = nc.dram_tensor("out", [N_SW * KOUT, B_CORE], bf16, kind="ExternalOutput").ap()

    with tile.TileContext(nc) as tc, ExitStack() as ctx:
        const = ctx.enter_context(tc.tile_pool(name="const", bufs=1))
        x_pool = ctx.enter_context(tc.tile_pool(name="xp", bufs=6))
        y_pool = ctx.enter_context(tc.tile_pool(name="yp", bufs=4))
        osb_pool = ctx.enter_context(tc.tile_pool(name="osb", bufs=4))
        pout_pool = ctx.enter_context(tc.tile_pool(name="pout", bufs=2, space="PSUM"))

        w_sb = const.tile([128, WB_COLS], bf16)
        ones = const.tile([1, B_CORE], bf16)
        nc.gpsimd.memset(ones[:], 1.0)
        # Three weight loads: [bias | s0 kt0-3] small and first on SP HWDGE
        # (so PE can start as soon as the first x chunk lands), the rest via
        # the Pool/SWDGE path — descriptor generation for the bulk weights
        # happens on the otherwise idle GPSIMD engine instead of the shared
        # HWDGE, which the 8 x chunks + 4 out DMAs already keep busy.
        wsplit = W0 + 4 * KOUT
        nc.sync.dma_start(out=w_sb[:, 0:wsplit], in_=w_ap[:, 0:wsplit])
        nc.gpsimd.dma_start(
            out=w_sb[:, wsplit : W0 + KT * KOUT], in_=w_ap[:, wsplit : W0 + KT * KOUT]
        )
        nc.gpsimd.dma_start(
            out=w_sb[:, W0 + KT * KOUT :], in_=w_ap[:, W0 + KT * KOUT :]
        )

        x_re = x_ap.rearrange("(t p) f -> p t f", p=128)  # [128, 32, 1024]

        out_dmas = []  # (engine, out slice, sbuf tile): issued after all x

        for s in range(N_SW):
            psum_out = pout_pool.tile([KOUT, B_CORE], f32)
            # bias via K=1 matmul against the ones row: starts the psum
            # accumulation group and keeps bias-add off the DVE epilogue.
            nc.tensor.matmul(
                psum_out[:],
                lhsT=w_sb[0:1, bass.ds(s * KOUT, KOUT)],
                rhs=ones[:],
                start=True,
                stop=False,
            )
            for g, (kt0, nkt) in enumerate(_chunk_plan(s)):
                xab = x_pool.tile([128, CHUNK_KT, DCT_NBINS * B_CORE], fp8)
                # alternate the two HWDGE queues (SP / ACT) for pipelined
                # descriptor generation while transfers serialize on HBM
                dma_eng = nc.scalar if (s * 4 + g) % 2 == 0 else nc.sync
                dma_eng.dma_start(
                    out=xab[:, 0:nkt, :], in_=x_re[:, bass.ds(s * KT + kt0, nkt), :]
                )
                kts = [kt0 + j for j in range(nkt)]
                presum = [kt for kt in kts if _presum_this(s, kt)]
                # DVE presum adds first so their y tiles are in flight...
                ys = {}
                for kt in presum:
                    y = y_pool.tile([128, B_CORE], bf16)
                    nc.vector.tensor_add(
                        y[:],
                        xab[:, kt - kt0, 0:B_CORE],
                        xab[:, kt - kt0, B_CORE:],
                    )
                    ys[kt] = y
                # ...then the direct matmuls (PE executes in order: a matmul
                # waiting on DVE must not block ready ones behind it), and
                # the presummed k-tiles' matmuls last.
                mms = [(kt, n) for kt in kts if kt not in ys for n in range(DCT_NBINS)]
                mms += [(kt, None) for kt in presum]
                for i, (kt, n) in enumerate(mms):
                    rhs = ys[kt][:] if n is None else xab[:, kt - kt0, bass.ts(n, B_CORE)]
                    nc.tensor.matmul(
                        psum_out[:],
                        lhsT=w_sb[:, bass.ds(W0 + (s * KT + kt) * KOUT, KOUT)],
                        rhs=rhs,
                        start=False,
                        stop=(g == len(_chunk_plan(s)) - 1 and i == len(mms) - 1),
                    )
            # epilogue: exact LeakyReLU as max(y, SLOPE*y) on DVE; bias
            # already in psum. Halved along batch so the first output DMA
            # starts early (GPSIMD cannot read PSUM, so both halves stay on
            # DVE). The DMAs themselves are deferred to the end of each
            # queue's program order — an out DMA queued between x chunks
            # would stall the x stream behind the epilogue's semaphore.
            for h in range(2):
                hb = bass.ts(h, B_CORE // 2)
                tl = osb_pool.tile([KOUT, B_CORE // 2], f32, tag="tl", name=f"tl_{s}_{h}")
                nc.vector.tensor_scalar_mul(tl[:], psum_out[:, hb], SLOPE)
                osb = osb_pool.tile([KOUT, B_CORE // 2], bf16, tag="osb", name=f"osb_{s}_{h}")
                nc.vector.tensor_max(osb[:], psum_out[:, hb], tl[:])
                eng = nc.sync if h == 0 else nc.scalar
                out_dmas.append((eng, out_ap[bass.ts(s, KOUT), hb], osb))

        for eng, dst, osb in out_dmas:
            eng.dma_start(out=dst, in_=osb[:])

    nc.compile()
    return nc


def _get_program():
    if "nc" not in _CACHE:
        _CACHE["nc"] = _build_program()
    return _CACHE["nc"]


def kernel(x, conv_w, conv_b):
    global LAST_RESULT
    shards = _shard_x(x)
    wb_host = _fold_weights(conv_w, conv_b)

    nc = _get_program()
    in_maps = [{"x": shards[c], "w": wb_host} for c in range(N_CORES)]
    trace = bool(int(os.environ.get("DCT_TRACE", "0")))
    res = run_bass_kernel_spmd(nc, in_maps, list(range(N_CORES)), trace=trace)
    LAST_RESULT = res
    # per-core output is [s*128+k, b] bf16; upcast + un-transpose during gather
    out = np.concatenate(
        [
            np.ascontiguousarray(np.asarray(res.results[c]["out"], np.float32).T)
            for c in range(N_CORES)
        ],
        axis=0,
    )
    return out
